# revision 17
# baseline (speedup 1.0000x reference)
"""Trainium2 Bass kernel for a complex-valued GRU cell.

Computes, for complex h = h_r + i*h_i, x = x_r + i*x_i (B=65536, D=256):
    z = sigmoid(h @ Wz_h + x @ Wz_x + bz)      (complex dense, complex sigmoid applied per-part)
    r = sigmoid(h @ Wr_h + x @ Wr_x + br)
    n = tanh((r*h) @ Wn_h + x @ Wn_x + bn)     (r*h complex elementwise)
    new_h = (1-z)*h + z*n  ==  h + z*(n - h)   (complex arithmetic)

Strategy (8 NeuronCores, data-parallel over batch):
  - Each core handles 8192 rows. Weights replicated.
  - All 24 real [8192,256]x[256,256] gemms run in bf16 with f32 PSUM accumulation.
    The four real gemm terms of each complex dense accumulate into one PSUM bank;
    minus signs are folded into pre-negated weight copies on the host.
  - Gemm form: out[b, n] = actT.T @ W, with activation-transpose chunks (via PE
    transpose) as the stationary operand and host-concatenated weight panels
    [W_gate_r | W_gate_i | ...] as the moving operand, so gate pre-activations
    land in natural [batch, feature] layout. Elementwise gates and final update
    then run in natural layout and outputs store directly, no output transpose.
"""

from contextlib import ExitStack

import ml_dtypes
import numpy as np

import concourse.bass as bass
import concourse.mybir as mybir
import concourse.tile as tile
from concourse import bacc
from concourse.bass_utils import run_bass_kernel_spmd
from concourse.masks import make_identity

B_TOTAL = 65536
D = 256
N_CORES = 8
ROWS = B_TOTAL // N_CORES  # 8192 rows per core
F = 4                      # 128-row chunks per round
_BF16 = ml_dtypes.bfloat16

_SIG = mybir.ActivationFunctionType.Sigmoid
_TANH = mybir.ActivationFunctionType.Tanh

_module_cache: dict = {}


def _build(rows: int, has_bias: bool) -> bass.Bass:
    f32 = mybir.dt.float32
    bf16 = mybir.dt.bfloat16
    n_rounds = rows // (F * 128)
    assert rows % (F * 128) == 0

    nc = bacc.Bacc("TRN2", target_bir_lowering=False, debug=False)

    d_in = {n: nc.dram_tensor(n, [rows, D], f32, kind="ExternalInput")
            for n in ("h_r", "h_i", "x_r", "x_i")}
    # All moving weight panels packed in one tensor (one DMA → one semaphore,
    # keeping per-matmul sync-wait counts within the ISA limit). Columns:
    # 4x 4D z/r panels [zpre_r|zpre_i|rpre_r|rpre_i] keyed by stationary
    # tensor (h_r, h_i, x_r, x_i), then 4x 2D n panels [npre_r|npre_i]
    # keyed by (rh_r, rh_i, x_r, x_i).
    W_COLS = 4 * 4 * D + 4 * 2 * D  # 6144
    d_w = nc.dram_tensor("w_all", [2, 128, W_COLS], bf16, kind="ExternalInput")
    if has_bias:
        d_bzr = nc.dram_tensor("b_zr", [1, 4 * D], f32, kind="ExternalInput")
        d_bn = nc.dram_tensor("b_n", [1, 2 * D], f32, kind="ExternalInput")
    d_out = {n: nc.dram_tensor(n, [rows, D], f32, kind="ExternalOutput")
             for n in ("out_r", "out_i")}

    CH = 128  # chunk edge

    with ExitStack() as ctx:
        tc = ctx.enter_context(tile.TileContext(nc))
        const = ctx.enter_context(tc.tile_pool(name="const", bufs=1))
        inp = ctx.enter_context(tc.tile_pool(name="inp", bufs=2))
        ttp = ctx.enter_context(tc.tile_pool(name="ttp", bufs=3))
        gates = ctx.enter_context(tc.tile_pool(name="gates", bufs=2))
        tmp = ctx.enter_context(tc.tile_pool(name="tmp", bufs=1))
        outp = ctx.enter_context(tc.tile_pool(name="outp", bufs=2))
        psum_ti = ctx.enter_context(tc.tile_pool(name="psum_ti", bufs=2, space="PSUM"))
        psum_tr = ctx.enter_context(tc.tile_pool(name="psum_tr", bufs=1, space="PSUM"))
        psum_g = ctx.enter_context(tc.tile_pool(name="psum_g", bufs=5, space="PSUM"))

        ident_f = const.tile([CH, CH], f32, tag="ident_f")
        make_identity(nc, ident_f)
        ident_b = const.tile([CH, CH], bf16, tag="ident_b")
        make_identity(nc, ident_b)

        w_all = const.tile([128, 2, W_COLS], bf16, tag="w_all")
        nc.sync.dma_start(out=w_all, in_=d_w.ap().rearrange("k p n -> p k n"))
        wa_off = {"h_r": 0, "h_i": 4 * D, "x_r": 8 * D, "x_i": 12 * D}
        wn_off = {"rh_r": 16 * D, "rh_i": 18 * D, "x_r": 20 * D, "x_i": 22 * D}

        if has_bias:
            b_zr = const.tile([128, 4 * D], f32, tag="b_zr")
            bcast = bass.AP(tensor=d_bzr, offset=0, ap=[[0, 128], [1, 4 * D]])
            nc.gpsimd.dma_start(out=b_zr, in_=bcast)
            b_n = const.tile([128, 2 * D], f32, tag="b_n")
            bcast = bass.AP(tensor=d_bn, offset=0, ap=[[0, 128], [1, 2 * D]])
            nc.gpsimd.dma_start(out=b_n, in_=bcast)

        for r in range(n_rounds):
            row0 = r * F * 128

            nat = {}
            for n in ("h_r", "h_i", "x_r", "x_i"):
                t = inp.tile([128, F, D], f32, tag=f"nat_{n}")
                nc.sync.dma_start(
                    out=t,
                    in_=d_in[n].ap()[row0:row0 + F * 128, :]
                    .rearrange("(c p) d -> p c d", p=128),
                )
                nat[n] = t

            # Transposed bf16 stationary copies. Per tensor: [128, 2F*128]
            # packing chunk (dk, c) at column (dk*F + c)*128. One PSUM bank
            # holds one dk-half ([128, F*128] f32 = 2KB).
            tT = {}
            for n in ("h_r", "h_i", "x_r", "x_i"):
                t = ttp.tile([128, 2 * F * CH], bf16, tag=f"tT_{n}")
                for dk in range(2):
                    ps = psum_ti.tile([128, F * CH], f32, tag="pst")
                    for c in range(F):
                        nc.tensor.transpose(
                            ps[:, c * CH:(c + 1) * CH],
                            nat[n][:, c, dk * CH:(dk + 1) * CH],
                            ident_f,
                        )
                    nc.any.tensor_copy(out=t[:, dk * F * CH:(dk + 1) * F * CH], in_=ps)
                tT[n] = t

            # z/r gemms: per batch chunk accumulate 8 matmuls into each of two
            # PSUM banks (z-pair, r-pair).
            z_sb = gates.tile([128, F, 2 * D], f32, tag="z")
            r_sb = gates.tile([128, F, 2 * D], f32, tag="r")
            for c in range(F):
                pz = psum_g.tile([128, 2 * D], f32, tag="pg")
                pr = psum_g.tile([128, 2 * D], f32, tag="pg")
                i = 0
                for n in ("h_r", "h_i", "x_r", "x_i"):
                    o = wa_off[n]
                    for dk in range(2):
                        lhsT = tT[n][:, (dk * F + c) * CH:(dk * F + c + 1) * CH]
                        first = i == 0
                        last = i == 7
                        nc.tensor.matmul(pz, lhsT, w_all[:, dk, o:o + 2 * D],
                                         start=first, stop=last)
                        nc.tensor.matmul(pr, lhsT, w_all[:, dk, o + 2 * D:o + 4 * D],
                                         start=first, stop=last)
                        i += 1
                if has_bias:
                    nc.vector.tensor_add(pz, pz, b_zr[:, 0:2 * D])
                    nc.vector.tensor_add(pr, pr, b_zr[:, 2 * D:4 * D])
                nc.scalar.activation(out=z_sb[:, c, :], in_=pz, func=_SIG)
                nc.scalar.activation(out=r_sb[:, c, :], in_=pr, func=_SIG)

            # rh = r * h (complex), emitted in bf16 for the n-phase gemms.
            # Runs on GPSIMD — DVE and ACT are the loaded engines.
            rr = r_sb[:, :, 0:D]
            ri = r_sb[:, :, D:2 * D]
            rh_r = tmp.tile([128, F, D], bf16, tag="rh_r")
            rh_i = tmp.tile([128, F, D], bf16, tag="rh_i")
            ta = tmp.tile([128, F, D], f32, tag="ta")
            tb = tmp.tile([128, F, D], f32, tag="tb")
            nc.vector.tensor_mul(ta, rr, nat["h_r"])
            nc.vector.tensor_mul(tb, ri, nat["h_i"])
            nc.vector.tensor_sub(rh_r, ta, tb)
            tc_ = tmp.tile([128, F, D], f32, tag="ta")
            td = tmp.tile([128, F, D], f32, tag="tb")
            nc.vector.tensor_mul(tc_, rr, nat["h_i"])
            nc.vector.tensor_mul(td, ri, nat["h_r"])
            nc.vector.tensor_add(rh_i, tc_, td)

            # Transpose rh (bf16). Same per-tensor packing as tT: chunk
            # (dk, c) at column (dk*F + c)*128; [128, 2F*128] bf16 = one bank.
            rhT = {}
            for tn, rh in (("rh_r", rh_r), ("rh_i", rh_i)):
                ps = psum_tr.tile([128, 2 * F * CH], bf16, tag="psr")
                for dk in range(2):
                    for c in range(F):
                        s = (dk * F + c) * CH
                        nc.tensor.transpose(
                            ps[:, s:s + CH],
                            rh[:, c, dk * CH:(dk + 1) * CH],
                            ident_b,
                        )
                t = ttp.tile([128, 2 * F * CH], bf16, tag=f"tT_{tn}")
                nc.any.tensor_copy(out=t, in_=ps)
                rhT[tn] = t

            # n gemms.
            n_sb = gates.tile([128, F, 2 * D], f32, tag="n")
            for c in range(F):
                pn = psum_g.tile([128, 2 * D], f32, tag="pg")
                i = 0
                for src, wkey in ((rhT, "rh_r"), (rhT, "rh_i"), (tT, "x_r"), (tT, "x_i")):
                    o = wn_off[wkey]
                    for dk in range(2):
                        lhsT = src[wkey][:, (dk * F + c) * CH:(dk * F + c + 1) * CH]
                        nc.tensor.matmul(pn, lhsT, w_all[:, dk, o:o + 2 * D],
                                         start=(i == 0), stop=(i == 7))
                        i += 1
                if has_bias:
                    nc.vector.tensor_add(pn, pn, b_n)
                nc.scalar.activation(out=n_sb[:, c, :], in_=pn, func=_TANH)

            # new_h = h + z*(n - h)   (complex)
            nr = n_sb[:, :, 0:D]
            ni = n_sb[:, :, D:2 * D]
            zr = z_sb[:, :, 0:D]
            zi = z_sb[:, :, D:2 * D]
            d_r = tmp.tile([128, F, D], f32, tag="d_r")
            d_i = tmp.tile([128, F, D], f32, tag="d_i")
            nc.vector.tensor_sub(d_r, nr, nat["h_r"])
            nc.vector.tensor_sub(d_i, ni, nat["h_i"])

            o_r = outp.tile([128, F, D], f32, tag="o_r")
            o_i = outp.tile([128, F, D], f32, tag="o_i")
            e1 = tmp.tile([128, F, D], f32, tag="ta")
            e2 = tmp.tile([128, F, D], f32, tag="tb")
            nc.vector.tensor_mul(e1, zr, d_r)
            nc.vector.tensor_mul(e2, zi, d_i)
            e3 = tmp.tile([128, F, D], f32, tag="e3")
            nc.vector.tensor_sub(e3, e1, e2)
            nc.vector.tensor_add(o_r, e3, nat["h_r"])
            e4 = tmp.tile([128, F, D], f32, tag="tg")
            e5 = tmp.tile([128, F, D], f32, tag="th")
            nc.gpsimd.tensor_mul(e4, zr, d_i)
            nc.gpsimd.tensor_mul(e5, zi, d_r)
            e6 = tmp.tile([128, F, D], f32, tag="ti")
            nc.gpsimd.tensor_add(e6, e4, e5)
            nc.gpsimd.tensor_add(o_i, e6, nat["h_i"])

            for n, o in (("out_r", o_r), ("out_i", o_i)):
                nc.sync.dma_start(
                    out=d_out[n].ap()[row0:row0 + F * 128, :]
                    .rearrange("(c p) d -> p c d", p=128),
                    in_=o,
                )

    nc.compile()
    return nc


def _get_module(rows: int, has_bias: bool) -> bass.Bass:
    key = (rows, has_bias)
    if key not in _module_cache:
        _module_cache[key] = _build(rows, has_bias)
    return _module_cache[key]


def _prep_weights(p: dict) -> np.ndarray:
    cat = lambda *ms: np.concatenate(ms, axis=1)
    w = cat(
        # z/r panels keyed by stationary tensor
        p["h_zr"], p["h_zi"], p["h_rr"], p["h_ri"],       # h_r
        -p["h_zi"], p["h_zr"], -p["h_ri"], p["h_rr"],     # h_i
        p["x_zr"], p["x_zi"], p["x_rr"], p["x_ri"],       # x_r
        -p["x_zi"], p["x_zr"], -p["x_ri"], p["x_rr"],     # x_i
        # n panels
        p["h_nr"], p["h_ni"],                             # rh_r
        -p["h_ni"], p["h_nr"],                            # rh_i
        p["x_nr"], p["x_ni"],                             # x_r
        -p["x_ni"], p["x_nr"],                            # x_i
    )  # [256, 6144]
    return np.ascontiguousarray(w.reshape(2, 128, 24 * D).astype(_BF16))


def make_in_maps(h_r, h_i, x_r, x_i, params, rows=ROWS, n_cores=N_CORES):
    p = {k: np.asarray(v, dtype=np.float32) for k, v in params.items()}
    bias_names = ("z_b_r", "z_b_i", "r_b_r", "r_b_i", "n_b_r", "n_b_i")
    has_bias = any(np.any(p[b] != 0.0) for b in bias_names)
    base = {"w_all": _prep_weights(p)}
    if has_bias:
        base["b_zr"] = np.ascontiguousarray(
            np.concatenate([p["z_b_r"], p["z_b_i"], p["r_b_r"], p["r_b_i"]])[None, :]
        ).astype(np.float32)
        base["b_n"] = np.ascontiguousarray(
            np.concatenate([p["n_b_r"], p["n_b_i"]])[None, :]
        ).astype(np.float32)
    ins = [np.ascontiguousarray(np.asarray(t, dtype=np.float32))
           for t in (h_r, h_i, x_r, x_i)]
    in_maps = []
    for c in range(n_cores):
        sl = slice(c * rows, (c + 1) * rows)
        m = dict(base)
        m["h_r"] = ins[0][sl]
        m["h_i"] = ins[1][sl]
        m["x_r"] = ins[2][sl]
        m["x_i"] = ins[3][sl]
        in_maps.append(m)
    return in_maps, has_bias


def kernel(h_r, h_i, x_r, x_i, params, _trace=False):
    in_maps, has_bias = make_in_maps(h_r, h_i, x_r, x_i, params)
    nc = _get_module(ROWS, has_bias)
    res = run_bass_kernel_spmd(nc, in_maps, core_ids=list(range(N_CORES)),
                               trace=_trace)
    out_r = np.concatenate([res.results[c]["out_r"] for c in range(N_CORES)], axis=0)
    out_i = np.concatenate([res.results[c]["out_i"] for c in range(N_CORES)], axis=0)
    kernel._last_results = res
    return (out_r, out_i)


# revision 18
# speedup vs baseline: 1.2285x; 1.2285x over previous
"""Trainium2 Bass kernel for a complex-valued GRU cell.

Computes, for complex h = h_r + i*h_i, x = x_r + i*x_i (B=65536, D=256):
    z = sigmoid(h @ Wz_h + x @ Wz_x + bz)      (complex dense, complex sigmoid applied per-part)
    r = sigmoid(h @ Wr_h + x @ Wr_x + br)
    n = tanh((r*h) @ Wn_h + x @ Wn_x + bn)     (r*h complex elementwise)
    new_h = (1-z)*h + z*n  ==  h + z*(n - h)   (complex arithmetic)

Strategy (8 NeuronCores, data-parallel over batch):
  - Each core handles 8192 rows. Weights replicated.
  - All 24 real [8192,256]x[256,256] gemms run in bf16 with f32 PSUM accumulation.
    The four real gemm terms of each complex dense accumulate into one PSUM bank;
    minus signs are folded into pre-negated weight copies on the host.
  - Gemm form: out[b, n] = actT.T @ W, with activation-transpose chunks (via PE
    transpose) as the stationary operand and host-concatenated weight panels
    [W_gate_r | W_gate_i | ...] as the moving operand, so gate pre-activations
    land in natural [batch, feature] layout. Elementwise gates and final update
    then run in natural layout and outputs store directly, no output transpose.
"""

from contextlib import ExitStack

import ml_dtypes
import numpy as np

import concourse.bass as bass
import concourse.mybir as mybir
import concourse.tile as tile
from concourse import bacc
from concourse.bass_utils import run_bass_kernel_spmd
from concourse.masks import make_identity

B_TOTAL = 65536
D = 256
N_CORES = 8
ROWS = B_TOTAL // N_CORES  # 8192 rows per core
F = 4                      # 128-row chunks per round
_BF16 = ml_dtypes.bfloat16

_SIG = mybir.ActivationFunctionType.Sigmoid
_TANH = mybir.ActivationFunctionType.Tanh

_module_cache: dict = {}


def _build(rows: int, has_bias: bool) -> bass.Bass:
    f32 = mybir.dt.float32
    bf16 = mybir.dt.bfloat16
    n_rounds = rows // (F * 128)
    assert rows % (F * 128) == 0

    nc = bacc.Bacc("TRN2", target_bir_lowering=False, debug=False)

    d_in = {n: nc.dram_tensor(n, [rows, D], f32, kind="ExternalInput")
            for n in ("h_r", "h_i", "x_r", "x_i")}
    # All moving weight panels packed in one tensor (one DMA → one semaphore,
    # keeping per-matmul sync-wait counts within the ISA limit). Columns:
    # 4x 4D z/r panels [zpre_r|zpre_i|rpre_r|rpre_i] keyed by stationary
    # tensor (h_r, h_i, x_r, x_i), then 4x 2D n panels [npre_r|npre_i]
    # keyed by (rh_r, rh_i, x_r, x_i).
    W_COLS = 4 * 4 * D + 4 * 2 * D  # 6144
    d_w = nc.dram_tensor("w_all", [2, 128, W_COLS], bf16, kind="ExternalInput")
    if has_bias:
        d_bzr = nc.dram_tensor("b_zr", [1, 4 * D], f32, kind="ExternalInput")
        d_bn = nc.dram_tensor("b_n", [1, 2 * D], f32, kind="ExternalInput")
    d_out = {n: nc.dram_tensor(n, [rows, D], f32, kind="ExternalOutput")
             for n in ("out_r", "out_i")}

    CH = 128  # chunk edge

    with ExitStack() as ctx:
        tc = ctx.enter_context(tile.TileContext(nc))
        const = ctx.enter_context(tc.tile_pool(name="const", bufs=1))
        inp = ctx.enter_context(tc.tile_pool(name="inp", bufs=2))
        ttp = ctx.enter_context(tc.tile_pool(name="ttp", bufs=3))
        gates = ctx.enter_context(tc.tile_pool(name="gates", bufs=2))
        tmp = ctx.enter_context(tc.tile_pool(name="tmp", bufs=1))
        outp = ctx.enter_context(tc.tile_pool(name="outp", bufs=2))
        psum_ti = ctx.enter_context(tc.tile_pool(name="psum_ti", bufs=2, space="PSUM"))
        psum_tr = ctx.enter_context(tc.tile_pool(name="psum_tr", bufs=1, space="PSUM"))
        psum_g = ctx.enter_context(tc.tile_pool(name="psum_g", bufs=5, space="PSUM"))

        ident_f = const.tile([CH, CH], f32, tag="ident_f")
        make_identity(nc, ident_f)
        ident_b = const.tile([CH, CH], bf16, tag="ident_b")
        make_identity(nc, ident_b)

        w_all = const.tile([128, 2, W_COLS], bf16, tag="w_all")
        nc.sync.dma_start(out=w_all, in_=d_w.ap().rearrange("k p n -> p k n"))
        wa_off = {"h_r": 0, "h_i": 4 * D, "x_r": 8 * D, "x_i": 12 * D}
        wn_off = {"rh_r": 16 * D, "rh_i": 18 * D, "x_r": 20 * D, "x_i": 22 * D}

        if has_bias:
            b_zr = const.tile([128, 4 * D], f32, tag="b_zr")
            bcast = bass.AP(tensor=d_bzr, offset=0, ap=[[0, 128], [1, 4 * D]])
            nc.gpsimd.dma_start(out=b_zr, in_=bcast)
            b_n = const.tile([128, 2 * D], f32, tag="b_n")
            bcast = bass.AP(tensor=d_bn, offset=0, ap=[[0, 128], [1, 2 * D]])
            nc.gpsimd.dma_start(out=b_n, in_=bcast)

        for r in range(n_rounds):
            row0 = r * F * 128

            nat = {}
            for n in ("h_r", "h_i", "x_r", "x_i"):
                t = inp.tile([128, F, D], f32, tag=f"nat_{n}")
                nc.sync.dma_start(
                    out=t,
                    in_=d_in[n].ap()[row0:row0 + F * 128, :]
                    .rearrange("(c p) d -> p c d", p=128),
                )
                nat[n] = t

            # Transposed bf16 stationary copies. Per tensor: [128, 2F*128]
            # packing chunk (dk, c) at column (dk*F + c)*128. One PSUM bank
            # holds one dk-half ([128, F*128] f32 = 2KB).
            tT = {}
            for n in ("h_r", "h_i", "x_r", "x_i"):
                t = ttp.tile([128, 2 * F * CH], bf16, tag=f"tT_{n}")
                for dk in range(2):
                    ps = psum_ti.tile([128, F * CH], f32, tag="pst")
                    for c in range(F):
                        nc.tensor.transpose(
                            ps[:, c * CH:(c + 1) * CH],
                            nat[n][:, c, dk * CH:(dk + 1) * CH],
                            ident_f,
                        )
                    nc.any.tensor_copy(out=t[:, dk * F * CH:(dk + 1) * F * CH], in_=ps)
                tT[n] = t

            # z/r gemms: per batch chunk accumulate 8 matmuls into each of two
            # PSUM banks (z-pair, r-pair).
            z_sb = gates.tile([128, F, 2 * D], f32, tag="z")
            r_sb = gates.tile([128, F, 2 * D], f32, tag="r")
            for c in range(F):
                pz = psum_g.tile([128, 2 * D], f32, tag="pg")
                pr = psum_g.tile([128, 2 * D], f32, tag="pg")
                i = 0
                for n in ("h_r", "h_i", "x_r", "x_i"):
                    o = wa_off[n]
                    for dk in range(2):
                        lhsT = tT[n][:, (dk * F + c) * CH:(dk * F + c + 1) * CH]
                        first = i == 0
                        last = i == 7
                        nc.tensor.matmul(pz, lhsT, w_all[:, dk, o:o + 2 * D],
                                         start=first, stop=last)
                        nc.tensor.matmul(pr, lhsT, w_all[:, dk, o + 2 * D:o + 4 * D],
                                         start=first, stop=last)
                        i += 1
                if has_bias:
                    nc.vector.tensor_add(pz, pz, b_zr[:, 0:2 * D])
                    nc.vector.tensor_add(pr, pr, b_zr[:, 2 * D:4 * D])
                nc.scalar.activation(out=z_sb[:, c, :], in_=pz, func=_SIG)
                nc.scalar.activation(out=r_sb[:, c, :], in_=pr, func=_SIG)

            # rh = r * h (complex), emitted in bf16 for the n-phase gemms.
            # Runs on GPSIMD — DVE and ACT are the loaded engines.
            rr = r_sb[:, :, 0:D]
            ri = r_sb[:, :, D:2 * D]
            rh_r = tmp.tile([128, F, D], bf16, tag="rh_r")
            rh_i = tmp.tile([128, F, D], bf16, tag="rh_i")
            ta = tmp.tile([128, F, D], f32, tag="ta")
            tb = tmp.tile([128, F, D], f32, tag="tb")
            nc.vector.tensor_mul(ta, rr, nat["h_r"])
            nc.vector.tensor_mul(tb, ri, nat["h_i"])
            nc.vector.tensor_sub(rh_r, ta, tb)
            tc_ = tmp.tile([128, F, D], f32, tag="ta")
            td = tmp.tile([128, F, D], f32, tag="tb")
            nc.vector.tensor_mul(tc_, rr, nat["h_i"])
            nc.vector.tensor_mul(td, ri, nat["h_r"])
            nc.vector.tensor_add(rh_i, tc_, td)

            # Transpose rh (bf16). Same per-tensor packing as tT: chunk
            # (dk, c) at column (dk*F + c)*128; [128, 2F*128] bf16 = one bank.
            rhT = {}
            for tn, rh in (("rh_r", rh_r), ("rh_i", rh_i)):
                ps = psum_tr.tile([128, 2 * F * CH], bf16, tag="psr")
                for dk in range(2):
                    for c in range(F):
                        s = (dk * F + c) * CH
                        nc.tensor.transpose(
                            ps[:, s:s + CH],
                            rh[:, c, dk * CH:(dk + 1) * CH],
                            ident_b,
                        )
                t = ttp.tile([128, 2 * F * CH], bf16, tag=f"tT_{tn}")
                nc.any.tensor_copy(out=t, in_=ps)
                rhT[tn] = t

            # n gemms.
            n_sb = gates.tile([128, F, 2 * D], f32, tag="n")
            for c in range(F):
                pn = psum_g.tile([128, 2 * D], f32, tag="pg")
                i = 0
                for src, wkey in ((rhT, "rh_r"), (rhT, "rh_i"), (tT, "x_r"), (tT, "x_i")):
                    o = wn_off[wkey]
                    for dk in range(2):
                        lhsT = src[wkey][:, (dk * F + c) * CH:(dk * F + c + 1) * CH]
                        nc.tensor.matmul(pn, lhsT, w_all[:, dk, o:o + 2 * D],
                                         start=(i == 0), stop=(i == 7))
                        i += 1
                if has_bias:
                    nc.vector.tensor_add(pn, pn, b_n)
                nc.scalar.activation(out=n_sb[:, c, :], in_=pn, func=_TANH)

            # new_h = h + z*(n - h)   (complex)
            nr = n_sb[:, :, 0:D]
            ni = n_sb[:, :, D:2 * D]
            zr = z_sb[:, :, 0:D]
            zi = z_sb[:, :, D:2 * D]
            d_r = tmp.tile([128, F, D], f32, tag="d_r")
            d_i = tmp.tile([128, F, D], f32, tag="d_i")
            nc.vector.tensor_sub(d_r, nr, nat["h_r"])
            nc.vector.tensor_sub(d_i, ni, nat["h_i"])

            o_r = outp.tile([128, F, D], f32, tag="o_r")
            o_i = outp.tile([128, F, D], f32, tag="o_i")
            e1 = tmp.tile([128, F, D], f32, tag="ta")
            e2 = tmp.tile([128, F, D], f32, tag="tb")
            nc.vector.tensor_mul(e1, zr, d_r)
            nc.vector.tensor_mul(e2, zi, d_i)
            e3 = tmp.tile([128, F, D], f32, tag="e3")
            nc.vector.tensor_sub(e3, e1, e2)
            nc.vector.tensor_add(o_r, e3, nat["h_r"])
            e4 = tmp.tile([128, F, D], f32, tag="tg")
            e5 = tmp.tile([128, F, D], f32, tag="th")
            nc.vector.tensor_mul(e4, zr, d_i)
            nc.vector.tensor_mul(e5, zi, d_r)
            e6 = tmp.tile([128, F, D], f32, tag="ti")
            nc.vector.tensor_add(e6, e4, e5)
            nc.vector.tensor_add(o_i, e6, nat["h_i"])

            for n, o in (("out_r", o_r), ("out_i", o_i)):
                nc.sync.dma_start(
                    out=d_out[n].ap()[row0:row0 + F * 128, :]
                    .rearrange("(c p) d -> p c d", p=128),
                    in_=o,
                )

    nc.compile()
    return nc


def _get_module(rows: int, has_bias: bool) -> bass.Bass:
    key = (rows, has_bias)
    if key not in _module_cache:
        _module_cache[key] = _build(rows, has_bias)
    return _module_cache[key]


def _prep_weights(p: dict) -> np.ndarray:
    cat = lambda *ms: np.concatenate(ms, axis=1)
    w = cat(
        # z/r panels keyed by stationary tensor
        p["h_zr"], p["h_zi"], p["h_rr"], p["h_ri"],       # h_r
        -p["h_zi"], p["h_zr"], -p["h_ri"], p["h_rr"],     # h_i
        p["x_zr"], p["x_zi"], p["x_rr"], p["x_ri"],       # x_r
        -p["x_zi"], p["x_zr"], -p["x_ri"], p["x_rr"],     # x_i
        # n panels
        p["h_nr"], p["h_ni"],                             # rh_r
        -p["h_ni"], p["h_nr"],                            # rh_i
        p["x_nr"], p["x_ni"],                             # x_r
        -p["x_ni"], p["x_nr"],                            # x_i
    )  # [256, 6144]
    return np.ascontiguousarray(w.reshape(2, 128, 24 * D).astype(_BF16))


def make_in_maps(h_r, h_i, x_r, x_i, params, rows=ROWS, n_cores=N_CORES):
    p = {k: np.asarray(v, dtype=np.float32) for k, v in params.items()}
    bias_names = ("z_b_r", "z_b_i", "r_b_r", "r_b_i", "n_b_r", "n_b_i")
    has_bias = any(np.any(p[b] != 0.0) for b in bias_names)
    base = {"w_all": _prep_weights(p)}
    if has_bias:
        base["b_zr"] = np.ascontiguousarray(
            np.concatenate([p["z_b_r"], p["z_b_i"], p["r_b_r"], p["r_b_i"]])[None, :]
        ).astype(np.float32)
        base["b_n"] = np.ascontiguousarray(
            np.concatenate([p["n_b_r"], p["n_b_i"]])[None, :]
        ).astype(np.float32)
    ins = [np.ascontiguousarray(np.asarray(t, dtype=np.float32))
           for t in (h_r, h_i, x_r, x_i)]
    in_maps = []
    for c in range(n_cores):
        sl = slice(c * rows, (c + 1) * rows)
        m = dict(base)
        m["h_r"] = ins[0][sl]
        m["h_i"] = ins[1][sl]
        m["x_r"] = ins[2][sl]
        m["x_i"] = ins[3][sl]
        in_maps.append(m)
    return in_maps, has_bias


def kernel(h_r, h_i, x_r, x_i, params, _trace=False):
    in_maps, has_bias = make_in_maps(h_r, h_i, x_r, x_i, params)
    nc = _get_module(ROWS, has_bias)
    res = run_bass_kernel_spmd(nc, in_maps, core_ids=list(range(N_CORES)),
                               trace=_trace)
    out_r = np.concatenate([res.results[c]["out_r"] for c in range(N_CORES)], axis=0)
    out_i = np.concatenate([res.results[c]["out_i"] for c in range(N_CORES)], axis=0)
    kernel._last_results = res
    return (out_r, out_i)
